# revision 1
# baseline (speedup 1.0000x reference)
"""MoNet (GMMConv GNN) distributed Trainium2 kernel.

Strategy (8 NeuronCores):
  - Nodes partitioned into 8 contiguous blocks of B=6250 (core m owns dests
    [m*B,(m+1)*B)).  Edges bucketed by destination core and sorted by dest, so
    each core's segment-sum over its dest block is fully local.
  - Per layer: each core computes its block of xg = h @ Wg (row-padded to 128
    cols), AllGather -> full xg table in DRAM, then per-edge gather of source
    rows via indirect DMA, gaussian-weighted segment-sum done as one-hot
    matmuls accumulating in PSUM (dest blocks of 128 nodes), fused with the
    root-weight matmul; epilogue relu+bias+residual in transposed layout.
  - Host does index prep only: degree/dinv, edge sorting/padding, per-core
    edge tables. All O(N*F) and O(E*F) math runs on device.
"""

import os
import sys
from contextlib import ExitStack

import numpy as np

if "/opt/trn_rl_repo" not in sys.path:
    sys.path.insert(0, "/opt/trn_rl_repo")

import concourse.bacc as bacc
import concourse.bass as bass
import concourse.mybir as mybir
import concourse.tile as tile
from concourse import bass_utils
from concourse.bass import IndirectOffsetOnAxis

F32 = mybir.dt.float32
I32 = mybir.dt.int32
AF = mybir.ActivationFunctionType
ALU = mybir.AluOpType

P = 128
EPS = 1e-15


class Cfg:
    def __init__(self, N=50000, E=800000, NFEAT=128, NHID=96, NCLASS=40, NL=2, C=8,
                 GCH=1):
        self.N, self.E, self.NFEAT, self.NHID, self.NCLASS = N, E, NFEAT, NCLASS and NCLASS, NCLASS
        self.NHID = NHID
        self.NL, self.C, self.GCH = NL, C, GCH
        assert N % C == 0
        self.B = N // C
        self.NBLK = (self.B + P - 1) // P
        self.USE_DG = True
        self.HALF = N // 2
        # xg table row padded to a 512B-multiple row (f32)
        self.XGW = ((NHID + 127) // 128) * 128


def host_prep(cfg, edge_index, edge_weight):
    """Sort/bucket edges by dest; build per-core padded edge tables."""
    N, C, B, NBLK = cfg.N, cfg.C, cfg.B, cfg.NBLK
    row = np.asarray(edge_index[0]).astype(np.int64)
    col = np.asarray(edge_index[1]).astype(np.int64)
    ew = np.asarray(edge_weight).astype(np.float64)
    deg = np.bincount(row, weights=ew, minlength=N).astype(np.float32)
    dinv = np.where(deg > 0, 1.0 / np.sqrt(deg.astype(np.float64)), 0.0).astype(np.float32)

    order = np.argsort(col, kind="stable")
    rs, cs = row[order], col[order]
    core = cs // B
    loc = cs - core * B
    blk = loc // P
    dl = (loc - blk * P).astype(np.float32)

    cnt = np.zeros((C, NBLK), np.int64)
    np.add.at(cnt, (core, blk), 1)
    tiles = ((cnt + P - 1) // P).max(axis=0)  # [NBLK] max over cores
    toff = np.concatenate([[0], np.cumsum(tiles)]).astype(np.int64)
    T = int(toff[-1])

    g = core * NBLK + blk
    gcnt = np.bincount(g, minlength=C * NBLK)
    gstart = np.concatenate([[0], np.cumsum(gcnt)])[:-1]
    idx_in_g = np.arange(len(g)) - gstart[g]
    lane = (idx_in_g % P).astype(np.int64)
    tcol = (toff[blk] + idx_in_g // P).astype(np.int64)

    srcA = np.zeros((C, P, T), np.int32)
    edA = np.zeros((C, P, 3 * T), np.float32)
    edA[:, :, 2 * T:3 * T] = -1.0  # dl sentinel: padded lanes never match iota
    srcA[core, lane, tcol] = rs
    edA[core, lane, tcol] = dinv[rs]
    edA[core, lane, T + tcol] = dinv[cs]
    edA[core, lane, 2 * T + tcol] = dl
    return dict(srcA=srcA, edA=edA, tiles=[int(t) for t in tiles], T=T)


def host_prep_dg(cfg, edge_index, edge_weight):
    """Edges bucketed by (dest block, source half) for int16 dma_gather."""
    N, C, B, NBLK, HALF = cfg.N, cfg.C, cfg.B, cfg.NBLK, cfg.HALF
    row = np.asarray(edge_index[0]).astype(np.int64)
    col = np.asarray(edge_index[1]).astype(np.int64)
    ew = np.asarray(edge_weight).astype(np.float64)
    deg = np.bincount(row, weights=ew, minlength=N).astype(np.float32)
    with np.errstate(divide="ignore"):
        dinv = np.where(deg > 0, 1.0 / np.sqrt(deg.astype(np.float64)), 0.0).astype(np.float32)

    half = (row >= HALF).astype(np.int64)
    core = col // B
    loc = col - core * B
    blk = loc // P
    order = np.lexsort((half, blk, core))
    rs, cs = row[order], col[order]
    hs = half[order]
    core, loc, blk = core[order], loc[order], blk[order]
    dl = (loc - blk * P).astype(np.float32)

    NG = NBLK * 2
    g = (blk * 2 + hs)  # group within core
    cnt = np.zeros((C, NG), np.int64)
    np.add.at(cnt, (core, g), 1)
    K = ((cnt + P - 1) // P).max(axis=0)  # [NG] tiles per (blk, half)
    toff = np.concatenate([[0], np.cumsum(K)]).astype(np.int64)
    T = int(toff[-1])

    gg = core * NG + g
    gcnt = np.bincount(gg, minlength=C * NG)
    gstart = np.concatenate([[0], np.cumsum(gcnt)])[:-1]
    idx_in_g = np.arange(len(gg)) - gstart[gg]
    lane = (idx_in_g % P).astype(np.int64)
    tloc = idx_in_g // P               # tile within the (blk, half) call
    tcol = (toff[g] + tloc).astype(np.int64)

    edA = np.zeros((C, P, 3 * T), np.float32)
    edA[:, :, 2 * T:3 * T] = -1.0
    edA[core, lane, tcol] = dinv[rs]
    edA[core, lane, T + tcol] = dinv[cs]
    edA[core, lane, 2 * T + tcol] = dl
    # int16 idx in wrapped-16 layout: flat k = tloc*128 + lane within a call;
    # element k at [k % 16, call_off*8 + k // 16]; pad = -1 (skipped).
    idxA = np.zeros((C, 16, 8 * T), np.int16)  # pad = row 0 (valid); sel kills it via dl=-1
    k = tloc * P + lane
    r16 = (k % 16).astype(np.int64)
    c16 = (toff[g] * 8 + k // 16).astype(np.int64)
    idxA[core, r16, c16] = (rs - hs * HALF).astype(np.int16)
    idxA = np.tile(idxA, (1, 8, 1))  # replicate 16-row block to 128 partitions
    calls = [(int(K[i]),) for i in range(NG)]
    return dict(idxA=idxA, edA=edA, K=[int(x) for x in K],
                toff=[int(x) for x in toff], T=T)


def build(cfg, prep, scal, dbg=False):
    """Build the SPMD Bass graph. scal: list of per-layer dicts with floats
    wp0, wp1, bp, neg_mu, s2inv."""
    NHID, NCLASS, NFEAT = cfg.NHID, cfg.NCLASS, cfg.NFEAT
    B, NBLK, NL, C, GCH, XGW = cfg.B, cfg.NBLK, cfg.NL, cfg.C, cfg.GCH, cfg.XGW
    T = prep["T"]
    HALF = cfg.HALF
    if cfg.USE_DG:
        K2, toff = prep["K"], prep["toff"]
        Kmax = max(max(K2), 1)
    else:
        tiles = prep["tiles"]

    nc = bacc.Bacc("TRN2", target_bir_lowering=False, debug=False, num_devices=C)
    hT_in = nc.declare_dram_parameter("hT", [NFEAT, B], F32, isOutput=False)
    if cfg.USE_DG:
        idx_in = nc.declare_dram_parameter("idx16", [P, 8 * T], mybir.dt.int16,
                                           isOutput=False)
    else:
        src_in = nc.declare_dram_parameter("src", [P, T], I32, isOutput=False)
    ed_in = nc.declare_dram_parameter("ed", [P, 3 * T], F32, isOutput=False)
    R_in = nc.declare_dram_parameter("R", [P, P], F32, isOutput=False)
    id_in = nc.declare_dram_parameter("ident", [P, P], F32, isOutput=False)
    Wemb_in = nc.declare_dram_parameter("Wemb", [NFEAT, NHID], F32, isOutput=False)
    Wg_in = nc.declare_dram_parameter("Wg", [NL, NHID, XGW], F32, isOutput=False)
    Wr_in = nc.declare_dram_parameter("Wr", [NL, NHID, NHID], F32, isOutput=False)
    Wo_in = nc.declare_dram_parameter("Wo", [NHID, NCLASS], F32, isOutput=False)
    bemb_in = nc.declare_dram_parameter("bemb", [NHID, 1], F32, isOutput=False)
    bconv_in = nc.declare_dram_parameter("bconv", [NHID, NL], F32, isOutput=False)
    bout_in = nc.declare_dram_parameter("bout", [P, NCLASS], F32, isOutput=False)
    out_ext = nc.declare_dram_parameter("out", [B, NCLASS], F32, isOutput=True)
    if dbg:
        dbg_xg = nc.declare_dram_parameter("dbg_xg", [cfg.N, XGW], F32, isOutput=True)
        dbg_h = nc.declare_dram_parameter("dbg_h", [NHID, B], F32, isOutput=True)
        dbg_xj = nc.declare_dram_parameter("dbg_xj", [P, 8 * XGW], F32, isOutput=True)

    from concourse import library_config

    with tile.TileContext(nc) as tc, ExitStack() as ctx:
        if cfg.USE_DG:
            nc.gpsimd.load_library(library_config.mlp)
        const = ctx.enter_context(tc.tile_pool(name="const", bufs=1))
        sbp = ctx.enter_context(tc.tile_pool(name="sbp", bufs=3))
        xjp = ctx.enter_context(tc.tile_pool(name="xjp", bufs=4))
        selp = ctx.enter_context(tc.tile_pool(name="selp", bufs=16))
        gp = ctx.enter_context(tc.tile_pool(name="gp", bufs=2))
        gaussp = ctx.enter_context(tc.tile_pool(name="gaussp", bufs=2))
        hp = ctx.enter_context(tc.tile_pool(name="hp", bufs=2))
        pag = ctx.enter_context(tc.tile_pool(name="pag", bufs=3, space="PSUM"))
        pmm = ctx.enter_context(tc.tile_pool(name="pmm", bufs=3, space="PSUM"))
        ptr = ctx.enter_context(tc.tile_pool(name="ptr", bufs=2, space="PSUM"))
        dramp = ctx.enter_context(tc.tile_pool(name="dramp", bufs=1, space="DRAM"))

        def cload(ap, shape, dtype=F32, name=None):
            t = const.tile(shape, dtype, name=name or "c")
            nc.sync.dma_start(out=t[:], in_=ap)
            return t

        hT_s = cload(hT_in[:, :], [NFEAT, B], name="hT_s")
        if cfg.USE_DG:
            idx_s = cload(idx_in[:, :], [P, 8 * T], mybir.dt.int16, name="idx_s")
        else:
            src_s = cload(src_in[:, :], [P, T], I32, name="src_s")
        ed_s = cload(ed_in[:, :], [P, 3 * T], name="ed_s")
        u_s = ed_s[:, 0:T]
        v_s = ed_s[:, T:2 * T]
        dl_s = ed_s[:, 2 * T:3 * T]
        R_s = cload(R_in[:, :], [P, P], name="R_s")
        id_s = cload(id_in[:, :], [P, P], name="id_s")
        Wemb_s = cload(Wemb_in[:, :], [NFEAT, NHID], name="Wemb_s")
        Wo_s = cload(Wo_in[:, :], [NHID, NCLASS], name="Wo_s")
        bemb_s = cload(bemb_in[:, :], [NHID, 1], name="bemb_s")
        bconv_s = cload(bconv_in[:, :], [NHID, NL], name="bconv_s")
        bout_s = cload(bout_in[:, :], [P, NCLASS], name="bout_s")
        Wg_s = const.tile([NHID, NL * XGW], F32, name="Wg_s")
        Wr_s = const.tile([NHID, NL * NHID], F32, name="Wr_s")
        for i in range(NL):
            nc.sync.dma_start(out=Wg_s[:, i * XGW:(i + 1) * XGW], in_=Wg_in[i])
            nc.sync.dma_start(out=Wr_s[:, i * NHID:(i + 1) * NHID], in_=Wr_in[i])
        Rv = const.tile([P, P], F32, name="Rv")
        nc.vector.tensor_copy(out=Rv[:], in_=R_s[:])
        bconv_a = const.tile([NHID, NL], F32, name="bconv_a")
        nc.scalar.copy(out=bconv_a[:], in_=bconv_s[:])
        bemb_a = const.tile([NHID, 1], F32, name="bemb_a")
        nc.scalar.copy(out=bemb_a[:], in_=bemb_s[:])
        bout_v = const.tile([P, NCLASS], F32, name="bout_v")
        nc.vector.tensor_copy(out=bout_v[:], in_=bout_s[:])

        def nodeblocks():
            for nt in range(NBLK):
                c0 = nt * P
                yield nt, c0, min(P, B - c0)

        # ---- embedding: h0_T[96, B] = (h @ Wemb + bemb).T ----
        h_cur = hp.tile([NHID, B], F32, tag="h", name="h0")
        for nt, c0, pn in nodeblocks():
            pe = pmm.tile([P, XGW], F32, tag="mm", name="pe")
            nc.tensor.matmul(pe[:pn, :NHID], lhsT=hT_s[:, c0:c0 + pn], rhs=Wemb_s[:],
                             start=True, stop=True)
            tmp = sbp.tile([P, NHID], F32, tag="embt", name="embt")
            nc.scalar.copy(out=tmp[:pn, :], in_=pe[:pn, :NHID])
            pt = ptr.tile([NHID, P], F32, tag="tr", name="pt")
            nc.tensor.transpose(out=pt[:, :pn], in_=tmp[:pn, :NHID], identity=id_s[:pn, :pn])
            nc.scalar.activation(out=h_cur[:, c0:c0 + pn], in_=pt[:, :pn],
                                 func=AF.Identity, bias=bemb_a[:, :1])

        # ---- layers ----
        for li in range(NL):
            sc = scal[li]
            # gaussian edge coefficients  [P, T]
            t1 = gp.tile([P, T], F32, tag="g1", name="g1")
            t2 = gp.tile([P, T], F32, tag="g2", name="g2")
            nc.vector.tensor_scalar(out=t1[:], in0=u_s[:], scalar1=sc["wp0"],
                                    scalar2=None, op0=ALU.mult)
            nc.vector.tensor_scalar(out=t2[:], in0=v_s[:], scalar1=sc["wp1"],
                                    scalar2=sc["bp"], op0=ALU.mult, op1=ALU.add)
            t3 = gp.tile([P, T], F32, tag="g1", name="g3")
            nc.vector.tensor_tensor(out=t3[:], in0=t1[:], in1=t2[:], op=ALU.add)
            t4 = gp.tile([P, T], F32, tag="g2", name="g4")
            nc.scalar.activation(out=t4[:], in_=t3[:], func=AF.Tanh)
            t4b = gp.tile([P, T], F32, tag="g1", name="g4b")
            nc.vector.tensor_scalar(out=t4b[:], in0=t4[:], scalar1=sc["neg_mu"],
                                    scalar2=None, op0=ALU.add)
            t5 = gp.tile([P, T], F32, tag="g2", name="g5")
            nc.scalar.activation(out=t5[:], in_=t4b[:], func=AF.Square)
            t6 = gp.tile([P, T], F32, tag="g1", name="g6")
            nc.scalar.activation(out=t6[:], in_=t5[:], func=AF.Exp, scale=sc["s2inv"])
            gauss_s = gaussp.tile([P, T], F32, tag="gauss", name="gauss")
            nc.vector.tensor_copy(out=gauss_s[:], in_=t6[:])

            # xg block + all-gather
            xg_src = dramp.tile([B, XGW], F32, tag="xgs", name=f"xg_src{li}")
            xg_full = dramp.tile([cfg.N, XGW], F32, tag="xgf", addr_space="Shared",
                                 name=f"xg_full{li}")
            for nt, c0, pn in nodeblocks():
                px = pmm.tile([P, XGW], F32, tag="mm", name="px")
                nc.tensor.matmul(px[:pn, :], lhsT=h_cur[:, c0:c0 + pn],
                                 rhs=Wg_s[:, li * XGW:(li + 1) * XGW],
                                 start=True, stop=True)
                xs = sbp.tile([P, XGW], F32, tag="xs", name="xs")
                nc.scalar.copy(out=xs[:pn, :], in_=px[:pn, :])
                nc.sync.dma_start(out=xg_src[c0:c0 + pn, :], in_=xs[:pn, :])
            nc.gpsimd.collective_compute(
                "AllGather", ALU.bypass,
                replica_groups=[list(range(C))],
                ins=[xg_src[:, :]],
                outs=[xg_full[:, :]],
            )

            if dbg and li == 0:
                nc.sync.dma_start(out=dbg_xg[:, :], in_=xg_full[:, :])
            # edge aggregation per dest block
            h_new = hp.tile([NHID, B], F32, tag="h", name=f"h{li + 1}")
            tg = 0
            for nt, c0, pn in nodeblocks():
                if cfg.USE_DG:
                    Tb = K2[2 * nt] + K2[2 * nt + 1]
                else:
                    Tb = tiles[nt]
                pa = pag.tile([P, NHID], F32, tag="pa", name="pa")
                nc.tensor.matmul(pa[:pn, :], lhsT=h_cur[:, c0:c0 + pn],
                                 rhs=Wr_s[:, li * NHID:(li + 1) * NHID],
                                 start=True, stop=(Tb == 0))
                if cfg.USE_DG:
                    tlast = tg + Tb - 1
                    for h in (0, 1):
                        Kh = K2[2 * nt + h]
                        if Kh == 0:
                            continue
                        off = toff[2 * nt + h]
                        xj = xjp.tile([P, Kmax * XGW], F32, tag="xj", name="xj")
                        MAXT = 7  # cap descriptors per call under the SWDGE ring size
                        for k0 in range(0, Kh, MAXT):
                            kc = min(MAXT, Kh - k0)
                            out_ap = xj[:, k0 * XGW:(k0 + kc) * XGW].rearrange(
                                "p (k e) -> p k e", e=XGW)
                            nc.gpsimd.dma_gather(
                                out_ap, xg_full[h * HALF:(h + 1) * HALF, :],
                                idx_s[:, (off + k0) * 8:(off + k0 + kc) * 8],
                                kc * P, kc * P, XGW)
                        for k in range(Kh):
                            t = off + k
                            sel = selp.tile([P, P], F32, tag="sel", name="sel")
                            nc.vector.tensor_scalar(
                                out=sel[:], in0=Rv[:],
                                scalar1=dl_s[:, t:t + 1], scalar2=gauss_s[:, t:t + 1],
                                op0=ALU.is_equal, op1=ALU.mult)
                            nc.tensor.matmul(pa[:pn, :], lhsT=sel[:, :pn],
                                             rhs=xj[:, k * XGW:k * XGW + NHID],
                                             start=False, stop=(t == tlast))
                else:
                    t0 = tg
                    while t0 < tg + Tb:
                        gn = min(GCH, tg + Tb - t0)
                        xj = xjp.tile([P, GCH * XGW], F32, tag="xj", name="xj")
                        nc.gpsimd.indirect_dma_start(
                            out=xj[:, :gn * XGW],
                            out_offset=None,
                            in_=xg_full[:, :],
                            in_offset=IndirectOffsetOnAxis(ap=src_s[:, t0:t0 + gn], axis=0),
                        )
                        if dbg and li == 0 and t0 == 0:
                            nc.sync.dma_start(out=dbg_xj[:, :gn * XGW], in_=xj[:, :gn * XGW])
                        for k in range(gn):
                            t = t0 + k
                            sel = selp.tile([P, P], F32, tag="sel", name="sel")
                            nc.vector.tensor_scalar(
                                out=sel[:], in0=Rv[:],
                                scalar1=dl_s[:, t:t + 1], scalar2=gauss_s[:, t:t + 1],
                                op0=ALU.is_equal, op1=ALU.mult)
                            nc.tensor.matmul(pa[:pn, :], lhsT=sel[:, :pn],
                                             rhs=xj[:, k * XGW:k * XGW + NHID],
                                             start=False, stop=(t == tg + Tb - 1))
                        t0 += gn
                # epilogue: h_new = h_cur + relu(agg + Wroot h + bconv)
                et = sbp.tile([P, NHID], F32, tag="et", name="et")
                nc.scalar.copy(out=et[:pn, :], in_=pa[:pn, :])
                pt2 = ptr.tile([NHID, P], F32, tag="tr", name="pt2")
                nc.tensor.transpose(out=pt2[:, :pn], in_=et[:pn, :NHID],
                                    identity=id_s[:pn, :pn])
                rl = sbp.tile([NHID, P], F32, tag="rl", name="rl")
                nc.scalar.activation(out=rl[:, :pn], in_=pt2[:, :pn], func=AF.Relu,
                                     bias=bconv_a[:, li:li + 1])
                nc.vector.tensor_tensor(out=h_new[:, c0:c0 + pn], in0=rl[:, :pn],
                                        in1=h_cur[:, c0:c0 + pn], op=ALU.add)
                tg += Tb
            h_cur = h_new
            if dbg and li == 0:
                nc.sync.dma_start(out=dbg_h[:, :], in_=h_cur[:, :])

        # ---- output head ----
        for nt, c0, pn in nodeblocks():
            po = pmm.tile([P, XGW], F32, tag="mm", name="po")
            nc.tensor.matmul(po[:pn, :NCLASS], lhsT=h_cur[:, c0:c0 + pn], rhs=Wo_s[:],
                             start=True, stop=True)
            ob = sbp.tile([P, NCLASS], F32, tag="ob", name="ob")
            nc.vector.tensor_tensor(out=ob[:pn, :], in0=po[:pn, :NCLASS],
                                    in1=bout_v[:pn, :], op=ALU.add)
            nc.sync.dma_start(out=out_ext[c0:c0 + pn, :], in_=ob[:pn, :])

    nc.finalize()
    return nc


def make_in_maps(cfg, prep, h, W_emb, b_emb, Wg, Wroot, b_conv, W_out, b_out):
    C, B, NL, NHID, XGW, NCLASS = cfg.C, cfg.B, cfg.NL, cfg.NHID, cfg.XGW, cfg.NCLASS
    h = np.asarray(h, np.float32)
    Wg_p = np.zeros((NL, NHID, XGW), np.float32)
    Wg_p[:, :, :NHID] = np.asarray(Wg, np.float32).reshape(NL, NHID, NHID)
    R = np.tile(np.arange(P, dtype=np.float32), (P, 1))
    ident = np.eye(P, dtype=np.float32)
    common = dict(
        R=np.ascontiguousarray(R),
        ident=np.ascontiguousarray(ident),
        Wemb=np.ascontiguousarray(np.asarray(W_emb, np.float32)),
        Wg=np.ascontiguousarray(Wg_p),
        Wr=np.ascontiguousarray(np.asarray(Wroot, np.float32)),
        Wo=np.ascontiguousarray(np.asarray(W_out, np.float32)),
        bemb=np.ascontiguousarray(np.asarray(b_emb, np.float32)[:, None]),
        bconv=np.ascontiguousarray(np.asarray(b_conv, np.float32).T),
        bout=np.ascontiguousarray(np.tile(np.asarray(b_out, np.float32), (P, 1))),
    )
    in_maps = []
    for m in range(C):
        d = dict(common)
        d["hT"] = np.ascontiguousarray(h[m * B:(m + 1) * B, :].T)
        if "idxA" in prep:
            d["idx16"] = np.ascontiguousarray(prep["idxA"][m])
        else:
            d["src"] = np.ascontiguousarray(prep["srcA"][m])
        d["ed"] = np.ascontiguousarray(prep["edA"][m])
        in_maps.append(d)
    return in_maps


def make_scal(cfg, Wp, bp, mu, sigma):
    Wp = np.asarray(Wp, np.float64)
    bp = np.asarray(bp, np.float64)
    mu = np.asarray(mu, np.float64)
    sigma = np.asarray(sigma, np.float64)
    out = []
    for i in range(cfg.NL):
        out.append(dict(
            wp0=float(Wp[i, 0, 0]),
            wp1=float(Wp[i, 1, 0]),
            bp=float(bp[i, 0]),
            neg_mu=float(-mu[i, 0, 0]),
            s2inv=float(-0.5 / (EPS + sigma[i, 0, 0] ** 2)),
        ))
    return out


def run(cfg, inputs, trace=False):
    hp_fn = host_prep_dg if cfg.USE_DG else host_prep
    prep = hp_fn(cfg, inputs["edge_index"], inputs["edge_weight"])
    scal = make_scal(cfg, inputs["Wp"], inputs["bp"], inputs["mu"], inputs["sigma"])
    nc = build(cfg, prep, scal)
    in_maps = make_in_maps(cfg, prep, inputs["h"], inputs["W_emb"], inputs["b_emb"],
                           inputs["Wg"], inputs["Wroot"], inputs["b_conv"],
                           inputs["W_out"], inputs["b_out"])
    res = bass_utils.run_bass_kernel_spmd(nc, in_maps, core_ids=list(range(cfg.C)),
                                          trace=trace)
    out = np.concatenate([res.results[m]["out"] for m in range(cfg.C)], axis=0)
    return out.astype(np.float32), res


def kernel(**inputs):
    cfg = Cfg()
    out, _ = run(cfg, inputs, trace=False)
    return out

